# revision 36
# baseline (speedup 1.0000x reference)
"""Trainium2 Bass kernel: causal multi-head attention block (B=2, T=2048, C=1024, H=16).

Sharding: 8 cores = 2 (batch) x 4 (head groups of 4 heads).  Each core computes
q/k/v projections for its 4 heads, causal attention, and a partial out-proj
(rows of wo for its head slice).  Host sums the 4 partials per batch element.

v3: single software-pipelined schedule, tuned for the HAM throttle (PE runs at
2.4 GHz only while "warm"; activity gaps drop it to 1.2 GHz).  Projection
matmuls for strip P+1 are interleaved into attention strip P's j-steps, and the
out-projection is deferred into the last attention strip so every phase keeps
the tensor engine near-fully busy.

Math notes:
  - scores scale 1/sqrt(64) folded into wq/bq on the host.
  - softmax without max-subtraction (scores are O(+-10) here; exp safe in fp32).
  - softmax denominator via a ones-column appended to v (row 64 of the AV PSUM).
  - normalization: copy denominator row out, broadcast it across partitions
    with an SBUF->SBUF DMA, one reciprocal_approx_fast, one in-place multiply.
  - causal masking: post-exp affine_select (upper-triangle keep) on gpsimd.
  - scores matmuls run K=64 (per-head q/k tiles); head A and head B of a pair
    occupy separate PSUM banks so both accumulations can use start=True.

Per-core layouts:
  xT      [1024, 2048]  x[b].T                          (bf16)
  wqkvT   [1024, 768]   [wq_s.T/8 | wk_s.T | wv_s.T]    (bf16)
  woT     [256, 1024]   wo[:, head_slice].T             (bf16)
  bqk     [128, 4]      cols: bq/8 (pair0,pair1), bk (pair0,pair1)
  bv_row  [1, 256], bo_row [1, 1024]
  y       [2048, 1024]  partial output (pre-sum)        (f32)
"""

import os
import sys

import numpy as np

try:
    import ml_dtypes
    BF16_NP = ml_dtypes.bfloat16
except ImportError:  # pragma: no cover
    BF16_NP = None

for _p in ("/opt/trn_rl_repo", "/root/.axon_site/_ro/trn_rl_repo"):
    if os.path.isdir(_p) and _p not in sys.path:
        sys.path.append(_p)

import concourse.bass as bass  # noqa: E402
import concourse.mybir as mybir  # noqa: E402
import concourse.tile as tile  # noqa: E402

F32 = mybir.dt.float32
BF16 = mybir.dt.bfloat16

B, T, C, H = 2, 2048, 1024, 16
D = C // H          # 64
HPC = 4             # heads per core
DPC = HPC * D       # 256 head-dims per core
NCORES = 8

CHUNK = 128         # s-chunk / contraction granularity
SST = 512           # attention t-strip == one PSUM bank of f32
VW = 2 * D          # 128 per head: [ones x64 | v x64] (PE-replicated denominator)

_CTRL_TYPES = (mybir.InstDrain, mybir.InstNoOp, mybir.InstEventSemaphore)


def split_excess_waits(nc, lim=1):
    """Walrus accepts at most one sync-wait per instruction; move extras onto
    same-engine NoOps inserted just before the owner."""
    k = 0
    for fn in nc.m.functions:
        for blk in fn.blocks:
            out = []
            changed = False
            for inst in blk.instructions:
                si = inst.sync_info
                if si is not None and si.on_wait and len(si.on_wait) > lim:
                    waits = list(si.on_wait)
                    extra, keep = waits[:-lim], waits[-lim:]
                    for w in extra:
                        nop = mybir.InstNoOp(name=f"waitfix_{k}", ins=[], outs=[])
                        k += 1
                        nop.engine = inst.engine
                        nop.sync_info = mybir.SyncInfo(on_wait=[w], on_update=[])
                        out.append(nop)
                    si.on_wait = keep
                    changed = True
                out.append(inst)
            if changed:
                blk.instructions = out
    return k


def build_nc(t_len=T, fix_waits=True):
    """Build the per-core SPMD Bass program (same program on all 8 cores)."""
    assert t_len % SST == 0
    nks = t_len // SST                # 4 strips
    n_cchunk = C // CHUNK             # 8
    n_ttile = t_len // CHUNK          # 16

    nc = bass.Bass(target_bir_lowering=False)

    xT = nc.dram_tensor("xT", [C, t_len], BF16, kind="ExternalInput")
    wqkvT = nc.dram_tensor("wqkvT", [C, 3 * DPC], BF16, kind="ExternalInput")
    woT = nc.dram_tensor("woT", [DPC, C], BF16, kind="ExternalInput")
    bqk = nc.dram_tensor("bqk", [CHUNK, 4], F32, kind="ExternalInput")
    bv_row = nc.dram_tensor("bv_row", [1, DPC], F32, kind="ExternalInput")
    y = nc.dram_tensor("y", [t_len, C], BF16, kind="ExternalOutput")

    Exp = mybir.ActivationFunctionType.Exp

    with tile.TileContext(nc) as tc:
        with tc.tile_pool(name="persist", bufs=1) as pp, \
             tc.tile_pool(name="work", bufs=1) as wp, \
             tc.tile_pool(name="dr", bufs=1, space="DRAM") as dr, \
             tc.tile_pool(name="ps", bufs=1, space="PSUM") as ps:
            # scalar-engine warmup: trigger the exp table load early
            warm = pp.tile([1, 8], F32, tag="warm", name="warm")
            nc.gpsimd.memset(warm, 1.0)
            nc.scalar.activation(warm, warm, mybir.ActivationFunctionType.Ln)
            nc.scalar.activation(warm, warm, Exp)

            # ---- input DMAs (spread across idle engine queues) ----
            bqk_sb = pp.tile([CHUNK, 4], F32, tag="bqk", name="bqk_sb")
            bv_bc = pp.tile([CHUNK, DPC], F32, tag="bv_bc", name="bv_bc")

            WW = 3 * DPC

            def load_xt(strip, parts, engs):
                """Load all 8 row-chunks of one x strip into a single wide
                tile via len(parts) rearranged DMAs; return per-chunk views."""
                xts = wp.tile([CHUNK, n_cchunk * SST], BF16, tag="xts", bufs=2,
                              name=f"xts_{strip}")
                c0 = 0
                for i, n in enumerate(parts):
                    engs[i % len(engs)].dma_start(
                        out=xts[:, c0 * SST:(c0 + n) * SST].rearrange(
                            "p (c t) -> p c t", t=SST),
                        in_=xT[c0 * CHUNK:(c0 + n) * CHUNK,
                               strip * SST:(strip + 1) * SST].rearrange(
                            "(c p) t -> p c t", p=CHUNK))
                    c0 += n
                return [xts[:, c * SST:(c + 1) * SST] for c in range(n_cchunk)]

            nc.gpsimd.dma_start(out=bqk_sb, in_=bqk[:, :])
            nc.gpsimd.dma_start(out=bv_bc, in_=bv_row[0:1, :].broadcast_to((CHUNK, DPC)))
            wts = pp.tile([CHUNK, n_cchunk * WW], BF16, tag="wts", name="wts")
            xts0 = wp.tile([CHUNK, n_cchunk * SST], BF16, tag="xts", bufs=2,
                           name="xts_0")
            rr_engs = [nc.gpsimd, nc.sync, nc.scalar]
            qi = 0
            for c in range(n_cchunk):
                rr_engs[qi % 3].dma_start(
                    out=wts[:, c * WW:(c + 1) * WW],
                    in_=wqkvT[c * CHUNK:(c + 1) * CHUNK, :])
                qi += 1
                rr_engs[qi % 3].dma_start(
                    out=xts0[:, c * SST:(c + 1) * SST],
                    in_=xT[c * CHUNK:(c + 1) * CHUNK, 0:SST])
                qi += 1
            w_sb = [wts[:, c * WW:(c + 1) * WW] for c in range(n_cchunk)]
            xt_bufs = {0: [xts0[:, c * SST:(c + 1) * SST]
                           for c in range(n_cchunk)]}
            if nks > 1:
                xt_bufs[1] = load_xt(1, [4, 4], [nc.sync, nc.scalar])

            wot = pp.tile([CHUNK, 2 * C], BF16, tag="wot", name="wot")
            wo_sb = [wot[:, 0:C], wot[:, C:2 * C]]

            def load_wo():
                nc.gpsimd.dma_start(
                    out=wot.rearrange("p (c w) -> p c w", w=C),
                    in_=woT[:, :].rearrange("(c p) w -> p c w", p=CHUNK))

            # HAM warm-up: keep the PE busy on throwaway matmuls while the
            # input DMAs land, so real work starts at 2.4 GHz
            dum = pp.tile([CHUNK, CHUNK], BF16, tag="dum", name="dum")
            nc.vector.memset(dum, 0.0)
            for i in range(96):
                pd = ps.tile([CHUNK, CHUNK], F32, tag="fill", bufs=2,
                             name=f"pd_{i}")
                nc.tensor.matmul(pd, dum, dum, start=True, stop=True)

            # ---- persistent activations ----
            qTh = [pp.tile([D, t_len], BF16, tag=f"qT{h}", name=f"qT{h}")
                   for h in range(4)]
            kTh = [pp.tile([D, t_len], BF16, tag=f"kT{h}", name=f"kT{h}")
                   for h in range(4)]
            vaug = [pp.tile([CHUNK, HPC * VW], BF16, tag=f"v{j}", name=f"v{j}")
                    for j in range(n_ttile)]
            for j in range(n_ttile):
                eng = nc.vector if j % 2 == 0 else nc.gpsimd
                eng.memset(vaug[j], 1.0)   # ones half survives; rest overwritten
            aoT = [pp.tile([CHUNK, t_len], BF16, tag=f"aoT{p}", name=f"aoT{p}")
                   for p in range(2)]

            # =========== emit helpers ===========
            def proj_qk_group(strip, pj, m, xt):
                pq = ps.tile([CHUNK, SST], F32, tag="fill", bufs=2,
                             name=f"pq{pj}{m}_{strip}")
                for c in range(n_cchunk):
                    nc.tensor.matmul(
                        pq,
                        w_sb[c][:, pj * DPC + m * CHUNK:pj * DPC + (m + 1) * CHUNK],
                        xt[c],
                        start=(c == 0), stop=(c == n_cchunk - 1))
                dst = kTh if pj else qTh
                for hf in range(2):
                    lo, hi = hf * D, (hf + 1) * D
                    nc.vector.tensor_scalar_add(
                        dst[2 * m + hf][:, strip * SST:(strip + 1) * SST],
                        pq[lo:hi, :],
                        bqk_sb[lo:hi, 2 * pj + m:2 * pj + m + 1])

            def proj_v_group(strip, u, xt):
                jt = 4 * strip + u
                pv = ps.tile([CHUNK, SST], F32, tag="fill", bufs=2,
                             name=f"pv_{jt}")
                for c in range(n_cchunk):
                    nc.tensor.matmul(
                        pv[:, 0:DPC],
                        xt[c][:, u * CHUNK:(u + 1) * CHUNK],
                        w_sb[c][:, 2 * DPC:3 * DPC],
                        start=(c == 0), stop=(c == n_cchunk - 1))
                nc.vector.tensor_add(
                    vaug[jt].rearrange("p (h e) -> p h e", e=VW)[:, :, D:2 * D],
                    pv[:, 0:DPC].rearrange("p (h d) -> p h d", d=D),
                    bv_bc.rearrange("p (h d) -> p h d", d=D))

            def outproj_group(jt, tail=False):
                for js in range(2):
                    py = ps.tile([CHUNK, SST], F32, tag="fill", bufs=2,
                                 name=f"py_{jt}_{js}")
                    for p in range(2):
                        nc.tensor.matmul(
                            py,
                            aoT[p][:, jt * CHUNK:(jt + 1) * CHUNK],
                            wo_sb[p][:, js * SST:(js + 1) * SST],
                            start=(p == 0), stop=(p == 1))
                    ysb = wp.tile([CHUNK, SST], BF16, tag="ysb", bufs=3,
                                  name=f"ysb_{jt}_{js}")
                    nc.vector.tensor_copy(ysb, py)
                    if tail:
                        eng = [nc.sync, nc.scalar][(2 * jt + js) % 2]
                    else:
                        eng = [nc.gpsimd, nc.sync][(2 * jt + js) % 2]
                    eng.dma_start(
                        out=y[jt * CHUNK:(jt + 1) * CHUNK,
                              js * SST:(js + 1) * SST],
                        in_=ysb)

            def jstep(p, ks, j, nj):
                """scores -> exp -> (mask) -> AV for one s-chunk j of strip ks.

                PSUM layout of sAB / av: head A in bank 0 (cols 0:512), head B
                in bank 1 (cols 512:1024) — each bank gets its own start=True.
                """
                hA, hB = 2 * p, 2 * p + 1
                off = max(0, CHUNK * j - SST * ks)
                L = SST - off
                t0 = SST * ks + off
                sAB = ps.tile([CHUNK, 2 * SST], F32, tag="sAB", bufs=2,
                              name=f"s_{p}_{ks}_{j}")
                nc.tensor.matmul(
                    sAB[:, 0:L],
                    kTh[hA][:, j * CHUNK:(j + 1) * CHUNK],
                    qTh[hA][:, t0:t0 + L],
                    start=True, stop=True)
                nc.tensor.matmul(
                    sAB[:, SST:SST + L],
                    kTh[hB][:, j * CHUNK:(j + 1) * CHUNK],
                    qTh[hB][:, t0:t0 + L],
                    start=True, stop=True)
                eAB = wp.tile([CHUNK, 2 * SST], BF16, tag="eAB", bufs=3,
                              name=f"e_{p}_{ks}_{j}")
                if L == SST:
                    nc.scalar.activation(eAB, sAB, Exp)
                else:
                    nc.scalar.activation(eAB[:, 0:L], sAB[:, 0:L], Exp)
                    nc.scalar.activation(eAB[:, SST:SST + L],
                                         sAB[:, SST:SST + L], Exp)
                if CHUNK * j >= SST * ks:  # diagonal tile: zero upper triangle
                    for base in (0, SST):
                        nc.gpsimd.affine_select(
                            out=eAB[:, base:base + CHUNK],
                            in_=eAB[:, base:base + CHUNK],
                            pattern=[[1, CHUNK]], channel_multiplier=-1,
                            base=0, compare_op=mybir.AluOpType.is_ge, fill=0.0)
                av = av_cur[0]
                nc.tensor.matmul(
                    av[:, off:SST],
                    vaug[j][:, hA * VW:(hA + 1) * VW],
                    eAB[:, 0:L],
                    start=(j == 0), stop=(j == nj - 1), skip_group_check=True)
                nc.tensor.matmul(
                    av[:, SST + off:2 * SST],
                    vaug[j][:, hB * VW:(hB + 1) * VW],
                    eAB[:, SST:SST + L],
                    start=(j == 0), stop=(j == nj - 1), skip_group_check=True)

            def normalize(p, ks, tail=False):
                """softmax-normalize strip ks of pair p into aoT[p].

                av rows 0:64 carry the PE-replicated denominator, 64:128 the
                unnormalized output."""
                av = av_cur[0]
                strip = slice(ks * SST, (ks + 1) * SST)
                if tail:
                    # low-latency path: 1/d = exp(-log d) on the (idle) scalar
                    rec = wp.tile([CHUNK, SST], F32, tag="rec", bufs=2,
                                  name=f"rec_{p}_{ks}")
                    nc.vector.tensor_copy(rec[0:D, :], av[0:D, 0:SST])
                    nc.vector.tensor_copy(rec[D:CHUNK, :], av[0:D, SST:2 * SST])
                    nc.vector.tensor_copy(aoT[p][0:D, strip],
                                          av[D:CHUNK, 0:SST])
                    nc.vector.tensor_copy(aoT[p][D:CHUNK, strip],
                                          av[D:CHUNK, SST:2 * SST])
                    nc.scalar.activation(rec, rec,
                                         mybir.ActivationFunctionType.Ln)
                    nc.scalar.activation(rec, rec, Exp, scale=-1.0)
                    nc.gpsimd.tensor_mul(aoT[p][:, strip], aoT[p][:, strip], rec)
                    return
                den = wp.tile([1, 2 * SST], F32, tag="den", bufs=2,
                              name=f"den_{p}_{ks}")
                # evict (unnormalized) + denominator row; frees the av banks
                nc.vector.tensor_copy(den, av[0:1, :])
                nc.vector.tensor_copy(aoT[p][0:D, strip],
                                      av[D:CHUNK, 0:SST])
                nc.scalar.activation(aoT[p][D:CHUNK, strip],
                                     av[D:CHUNK, SST:2 * SST],
                                     mybir.ActivationFunctionType.Copy)
                # reciprocal via a [128, 8] reshape (wide on the DVE lanes),
                # then partition-broadcast — both through small DRAM bounces
                # (SBUF APs cannot reshape across partitions / stride-0 bcast)
                dden = dr.tile([1, 2 * SST], F32, tag="dden", bufs=2,
                               name=f"dden_{p}_{ks}")
                nc.sync.dma_start(out=dden, in_=den)
                nf = 2 * SST // CHUNK
                dsb = wp.tile([CHUNK, nf], F32, tag="dsb", bufs=2,
                              name=f"dsb_{p}_{ks}")
                dview = dden.rearrange("a b -> (a b)").rearrange(
                    "(p f) -> p f", p=CHUNK)
                nc.sync.dma_start(out=dsb, in_=dview)
                rsb = wp.tile([CHUNK, nf], F32, tag="rsb", bufs=2,
                              name=f"rsb_{p}_{ks}")
                nc.vector.reciprocal(rsb, dsb)
                drec = dr.tile([1, 2 * SST], F32, tag="drec", bufs=2,
                               name=f"drec_{p}_{ks}")
                rview = drec.rearrange("a b -> (a b)").rearrange(
                    "(p f) -> p f", p=CHUNK)
                nc.sync.dma_start(out=rview, in_=rsb)
                bcf = wp.tile([CHUNK, SST], F32, tag="bcf", bufs=2,
                              name=f"bcf_{p}_{ks}")
                nc.sync.dma_start(
                    out=bcf[0:D, :],
                    in_=drec[0:1, 0:SST].broadcast_to((D, SST)))
                nc.sync.dma_start(
                    out=bcf[D:CHUNK, :],
                    in_=drec[0:1, SST:2 * SST].broadcast_to((D, SST)))
                nc.gpsimd.tensor_mul(aoT[p][:, strip], aoT[p][:, strip], bcf)

            # =========== schedule ===========
            # pre-loop: full projection of strip 0
            for pj in (1, 0):
                for m in range(2):
                    proj_qk_group(0, pj, m, xt_bufs[0])
            for u in range(4):
                proj_v_group(0, u, xt_bufs[0])

            av_cur = [None]
            for ks in range(nks):
                if ks == 0:
                    load_wo()
                # prefetch x strip ks+2 (its buffer was freed by proj(ks))
                if ks + 2 < nks:
                    xt_bufs[ks + 2] = load_xt(ks + 2, [4, 4], [nc.sync])

                fillers = []
                if ks + 1 < nks:                  # projection of next strip
                    xt = xt_bufs[ks + 1]
                    for pj in (1, 0):
                        for m in range(2):
                            fillers.append((proj_qk_group, (ks + 1, pj, m, xt)))
                    for u in range(4):
                        fillers.append((proj_v_group, (ks + 1, u, xt)))
                if ks == nks - 1:                 # deferred out-proj (strips 0..ks-1)
                    for jt in range(4 * (nks - 1)):
                        fillers.append((outproj_group, (jt,)))

                nj = 4 * ks + 4
                total_js = 2 * nj
                gi = 0
                cnt = 0
                for p in (0, 1):
                    av_cur[0] = ps.tile([CHUNK, 2 * SST], F32, tag="av", bufs=1,
                                        name=f"av_{p}_{ks}")
                    for j in range(nj):
                        jstep(p, ks, j, nj)
                        cnt += 1
                        want = (cnt * len(fillers)) // total_js
                        while gi < want:
                            fn, args = fillers[gi]
                            fn(*args)
                            gi += 1
                    normalize(p, ks, tail=(ks == nks - 1 and p == 1))
                    # keep the tensor queue fed while av drains
                    if gi < len(fillers):
                        fn, args = fillers[gi]
                        fn(*args)
                        gi += 1
                while gi < len(fillers):
                    fn, args = fillers[gi]
                    fn(*args)
                    gi += 1

            # tail: out-proj of the last strip
            for jt in range(4 * (nks - 1), 4 * nks):
                outproj_group(jt, tail=True)

    if fix_waits:
        split_excess_waits(nc)
    return nc


def make_in_maps(x, wq, bq, wk, bk, wv, bv, wo, bo, t_len=T):
    """Build the 8 per-core input dicts from full inputs."""
    in_maps = []
    scale = 1.0 / np.sqrt(np.float32(D))
    for core in range(NCORES):
        b, hg = core // 4, core % 4
        sl = slice(DPC * hg, DPC * (hg + 1))
        wqs = (wq[sl] * scale).astype(np.float32)
        bqs = (bq[sl] * scale).astype(np.float32)
        wqkvT = np.concatenate([wqs.T, wk[sl].T, wv[sl].T], axis=1)
        bqk = np.stack([bqs[0:CHUNK], bqs[CHUNK:2 * CHUNK],
                        bk[sl][0:CHUNK], bk[sl][CHUNK:2 * CHUNK]], axis=1)
        in_maps.append({
            "xT": np.ascontiguousarray(x[b, :t_len].T).astype(BF16_NP),
            "wqkvT": np.ascontiguousarray(wqkvT).astype(BF16_NP),
            "woT": np.ascontiguousarray(wo[:, sl].T).astype(BF16_NP),
            "bqk": np.ascontiguousarray(bqk, dtype=np.float32),
            "bv_row": np.ascontiguousarray(bv[sl][None, :], dtype=np.float32),
        })
    return in_maps


def gather_output(results, bo, t_len=T):
    ys = [np.asarray(results[i]["y"], dtype=np.float32) for i in range(NCORES)]
    out = np.stack([ys[0] + ys[1] + ys[2] + ys[3],
                    ys[4] + ys[5] + ys[6] + ys[7]])
    out += np.asarray(bo, np.float32)[None, None, :]
    return out


_NC_CACHE = {}


def _get_nc(t_len=T):
    if t_len not in _NC_CACHE:
        _NC_CACHE[t_len] = build_nc(t_len)
    return _NC_CACHE[t_len]


def kernel(x, wq, bq, wk, bk, wv, bv, wo, bo, mask=None, **_unused):
    """Full-input entry point: shard, run on 8 NeuronCores, gather."""
    from concourse.bass_utils import run_bass_kernel_spmd

    x = np.asarray(x, dtype=np.float32)
    in_maps = make_in_maps(x, np.asarray(wq, np.float32), np.asarray(bq, np.float32),
                           np.asarray(wk, np.float32), np.asarray(bk, np.float32),
                           np.asarray(wv, np.float32), np.asarray(bv, np.float32),
                           np.asarray(wo, np.float32), np.asarray(bo, np.float32))
    nc = _get_nc(T)
    res = run_bass_kernel_spmd(nc, in_maps, list(range(NCORES)))
    return gather_output(res.results, bo)


# revision 37
# speedup vs baseline: 1.1642x; 1.1642x over previous
"""Trainium2 Bass kernel: causal multi-head attention block (B=2, T=2048, C=1024, H=16).

Sharding: 8 cores = 2 (batch) x 4 (head groups of 4 heads).  Each core computes
q/k/v projections for its 4 heads, causal attention, and a partial out-proj
(rows of wo for its head slice).  Host sums the 4 partials per batch element.

v3: single software-pipelined schedule, tuned for the HAM throttle (PE runs at
2.4 GHz only while "warm"; activity gaps drop it to 1.2 GHz).  Projection
matmuls for strip P+1 are interleaved into attention strip P's j-steps, and the
out-projection is deferred into the last attention strip so every phase keeps
the tensor engine near-fully busy.

Math notes:
  - scores scale 1/sqrt(64) folded into wq/bq on the host.
  - softmax without max-subtraction (scores are O(+-10) here; exp safe in fp32).
  - softmax denominator via a ones-column appended to v (row 64 of the AV PSUM).
  - normalization: copy denominator row out, broadcast it across partitions
    with an SBUF->SBUF DMA, one reciprocal_approx_fast, one in-place multiply.
  - causal masking: post-exp affine_select (upper-triangle keep) on gpsimd.
  - scores matmuls run K=64 (per-head q/k tiles); head A and head B of a pair
    occupy separate PSUM banks so both accumulations can use start=True.

Per-core layouts:
  xT      [1024, 2048]  x[b].T                          (bf16)
  wqkvT   [1024, 768]   [wq_s.T/8 | wk_s.T | wv_s.T]    (bf16)
  woT     [256, 1024]   wo[:, head_slice].T             (bf16)
  bqk     [128, 4]      cols: bq/8 (pair0,pair1), bk (pair0,pair1)
  bv_row  [1, 256], bo_row [1, 1024]
  y       [2048, 1024]  partial output (pre-sum)        (f32)
"""

import os
import sys

import numpy as np

try:
    import ml_dtypes
    BF16_NP = ml_dtypes.bfloat16
except ImportError:  # pragma: no cover
    BF16_NP = None

for _p in ("/opt/trn_rl_repo", "/root/.axon_site/_ro/trn_rl_repo"):
    if os.path.isdir(_p) and _p not in sys.path:
        sys.path.append(_p)

import concourse.bass as bass  # noqa: E402
import concourse.mybir as mybir  # noqa: E402
import concourse.tile as tile  # noqa: E402

F32 = mybir.dt.float32
BF16 = mybir.dt.bfloat16

B, T, C, H = 2, 2048, 1024, 16
D = C // H          # 64
HPC = 4             # heads per core
DPC = HPC * D       # 256 head-dims per core
NCORES = 8

CHUNK = 128         # s-chunk / contraction granularity
SST = 512           # attention t-strip == one PSUM bank of f32
VW = 2 * D          # 128 per head: [ones x64 | v x64] (PE-replicated denominator)

_CTRL_TYPES = (mybir.InstDrain, mybir.InstNoOp, mybir.InstEventSemaphore)


def split_excess_waits(nc, lim=1):
    """Walrus accepts at most one sync-wait per instruction; move extras onto
    same-engine NoOps inserted just before the owner."""
    k = 0
    for fn in nc.m.functions:
        for blk in fn.blocks:
            out = []
            changed = False
            for inst in blk.instructions:
                si = inst.sync_info
                if si is not None and si.on_wait and len(si.on_wait) > lim:
                    waits = list(si.on_wait)
                    extra, keep = waits[:-lim], waits[-lim:]
                    for w in extra:
                        nop = mybir.InstNoOp(name=f"waitfix_{k}", ins=[], outs=[])
                        k += 1
                        nop.engine = inst.engine
                        nop.sync_info = mybir.SyncInfo(on_wait=[w], on_update=[])
                        out.append(nop)
                    si.on_wait = keep
                    changed = True
                out.append(inst)
            if changed:
                blk.instructions = out
    return k


def build_nc(t_len=T, fix_waits=True):
    """Build the per-core SPMD Bass program (same program on all 8 cores)."""
    assert t_len % SST == 0
    nks = t_len // SST                # 4 strips
    n_cchunk = C // CHUNK             # 8
    n_ttile = t_len // CHUNK          # 16

    nc = bass.Bass(target_bir_lowering=False)

    xT = nc.dram_tensor("xT", [C, t_len], BF16, kind="ExternalInput")
    wqkvT = nc.dram_tensor("wqkvT", [C, 3 * DPC], BF16, kind="ExternalInput")
    woT = nc.dram_tensor("woT", [DPC, C], BF16, kind="ExternalInput")
    bqk = nc.dram_tensor("bqk", [CHUNK, 4], F32, kind="ExternalInput")
    bv_row = nc.dram_tensor("bv_row", [1, DPC], F32, kind="ExternalInput")
    y = nc.dram_tensor("y", [t_len, C], BF16, kind="ExternalOutput")

    Exp = mybir.ActivationFunctionType.Exp

    with tile.TileContext(nc) as tc:
        with tc.tile_pool(name="persist", bufs=1) as pp, \
             tc.tile_pool(name="work", bufs=1) as wp, \
             tc.tile_pool(name="dr", bufs=1, space="DRAM") as dr, \
             tc.tile_pool(name="ps", bufs=1, space="PSUM") as ps:
            # scalar-engine warmup: trigger the exp table load early
            warm = pp.tile([1, 8], F32, tag="warm", name="warm")
            nc.gpsimd.memset(warm, 1.0)
            nc.scalar.activation(warm, warm, mybir.ActivationFunctionType.Ln)
            nc.scalar.activation(warm, warm, Exp)

            # ---- input DMAs (spread across idle engine queues) ----
            bqk_sb = pp.tile([CHUNK, 4], F32, tag="bqk", name="bqk_sb")
            bv_bc = pp.tile([CHUNK, DPC], F32, tag="bv_bc", name="bv_bc")

            WW = 3 * DPC

            def load_xt(strip, parts, engs):
                """Load all 8 row-chunks of one x strip into a single wide
                tile via len(parts) rearranged DMAs; return per-chunk views."""
                xts = wp.tile([CHUNK, n_cchunk * SST], BF16, tag="xts", bufs=2,
                              name=f"xts_{strip}")
                c0 = 0
                for i, n in enumerate(parts):
                    engs[i % len(engs)].dma_start(
                        out=xts[:, c0 * SST:(c0 + n) * SST].rearrange(
                            "p (c t) -> p c t", t=SST),
                        in_=xT[c0 * CHUNK:(c0 + n) * CHUNK,
                               strip * SST:(strip + 1) * SST].rearrange(
                            "(c p) t -> p c t", p=CHUNK))
                    c0 += n
                return [xts[:, c * SST:(c + 1) * SST] for c in range(n_cchunk)]

            nc.gpsimd.dma_start(out=bqk_sb, in_=bqk[:, :])
            nc.gpsimd.dma_start(out=bv_bc, in_=bv_row[0:1, :].broadcast_to((CHUNK, DPC)))
            wts = pp.tile([CHUNK, n_cchunk * WW], BF16, tag="wts", name="wts")
            xts0 = wp.tile([CHUNK, n_cchunk * SST], BF16, tag="xts", bufs=2,
                           name="xts_0")
            rr_engs = [nc.gpsimd, nc.sync, nc.scalar]
            qi = 0
            for c in range(n_cchunk):
                rr_engs[qi % 3].dma_start(
                    out=wts[:, c * WW:(c + 1) * WW],
                    in_=wqkvT[c * CHUNK:(c + 1) * CHUNK, :])
                qi += 1
                rr_engs[qi % 3].dma_start(
                    out=xts0[:, c * SST:(c + 1) * SST],
                    in_=xT[c * CHUNK:(c + 1) * CHUNK, 0:SST])
                qi += 1
            w_sb = [wts[:, c * WW:(c + 1) * WW] for c in range(n_cchunk)]
            xt_bufs = {0: [xts0[:, c * SST:(c + 1) * SST]
                           for c in range(n_cchunk)]}
            if nks > 1:
                xt_bufs[1] = load_xt(1, [4, 4], [nc.sync, nc.scalar])

            wot = pp.tile([CHUNK, 2 * C], BF16, tag="wot", name="wot")
            wo_sb = [wot[:, 0:C], wot[:, C:2 * C]]

            def load_wo():
                nc.gpsimd.dma_start(
                    out=wot.rearrange("p (c w) -> p c w", w=C),
                    in_=woT[:, :].rearrange("(c p) w -> p c w", p=CHUNK))

            # HAM warm-up: keep the PE busy on throwaway matmuls while the
            # input DMAs land, so real work starts at 2.4 GHz
            dum = pp.tile([CHUNK, SST], BF16, tag="dum", name="dum")
            nc.vector.memset(dum, 0.0)
            for i in range(10):
                pd = ps.tile([CHUNK, SST], F32, tag="fill", bufs=2,
                             name=f"pd_{i}")
                nc.tensor.matmul(pd, dum[:, 0:CHUNK], dum, start=True, stop=True)

            # ---- persistent activations ----
            qTh = [pp.tile([D, t_len], BF16, tag=f"qT{h}", name=f"qT{h}")
                   for h in range(4)]
            kTh = [pp.tile([D, t_len], BF16, tag=f"kT{h}", name=f"kT{h}")
                   for h in range(4)]
            vaug = [pp.tile([CHUNK, HPC * VW], BF16, tag=f"v{j}", name=f"v{j}")
                    for j in range(n_ttile)]
            for j in range(n_ttile):
                eng = nc.vector if j % 2 == 0 else nc.gpsimd
                eng.memset(vaug[j], 1.0)   # ones half survives; rest overwritten
            aoT = [pp.tile([CHUNK, t_len], BF16, tag=f"aoT{p}", name=f"aoT{p}")
                   for p in range(2)]

            # =========== emit helpers ===========
            def proj_qk_group(strip, pj, m, xt):
                pq = ps.tile([CHUNK, SST], F32, tag="fill", bufs=2,
                             name=f"pq{pj}{m}_{strip}")
                for c in range(n_cchunk):
                    nc.tensor.matmul(
                        pq,
                        w_sb[c][:, pj * DPC + m * CHUNK:pj * DPC + (m + 1) * CHUNK],
                        xt[c],
                        start=(c == 0), stop=(c == n_cchunk - 1))
                dst = kTh if pj else qTh
                for hf in range(2):
                    lo, hi = hf * D, (hf + 1) * D
                    nc.vector.tensor_scalar_add(
                        dst[2 * m + hf][:, strip * SST:(strip + 1) * SST],
                        pq[lo:hi, :],
                        bqk_sb[lo:hi, 2 * pj + m:2 * pj + m + 1])

            def proj_v_group(strip, u, xt):
                jt = 4 * strip + u
                pv = ps.tile([CHUNK, SST], F32, tag="fill", bufs=2,
                             name=f"pv_{jt}")
                for c in range(n_cchunk):
                    nc.tensor.matmul(
                        pv[:, 0:DPC],
                        xt[c][:, u * CHUNK:(u + 1) * CHUNK],
                        w_sb[c][:, 2 * DPC:3 * DPC],
                        start=(c == 0), stop=(c == n_cchunk - 1))
                nc.vector.tensor_add(
                    vaug[jt].rearrange("p (h e) -> p h e", e=VW)[:, :, D:2 * D],
                    pv[:, 0:DPC].rearrange("p (h d) -> p h d", d=D),
                    bv_bc.rearrange("p (h d) -> p h d", d=D))

            def outproj_group(jt, tail=False):
                for js in range(2):
                    py = ps.tile([CHUNK, SST], F32, tag="fill", bufs=2,
                                 name=f"py_{jt}_{js}")
                    for p in range(2):
                        nc.tensor.matmul(
                            py,
                            aoT[p][:, jt * CHUNK:(jt + 1) * CHUNK],
                            wo_sb[p][:, js * SST:(js + 1) * SST],
                            start=(p == 0), stop=(p == 1))
                    ysb = wp.tile([CHUNK, SST], BF16, tag="ysb", bufs=3,
                                  name=f"ysb_{jt}_{js}")
                    nc.vector.tensor_copy(ysb, py)
                    if tail:
                        eng = [nc.sync, nc.scalar, nc.gpsimd][(2 * jt + js) % 3]
                    else:
                        eng = [nc.gpsimd, nc.sync][(2 * jt + js) % 2]
                    eng.dma_start(
                        out=y[jt * CHUNK:(jt + 1) * CHUNK,
                              js * SST:(js + 1) * SST],
                        in_=ysb)

            def jstep(p, ks, j, nj):
                """scores -> exp -> (mask) -> AV for one s-chunk j of strip ks.

                PSUM layout of sAB / av: head A in bank 0 (cols 0:512), head B
                in bank 1 (cols 512:1024) — each bank gets its own start=True.
                """
                hA, hB = 2 * p, 2 * p + 1
                off = max(0, CHUNK * j - SST * ks)
                L = SST - off
                t0 = SST * ks + off
                sAB = ps.tile([CHUNK, 2 * SST], F32, tag="sAB", bufs=2,
                              name=f"s_{p}_{ks}_{j}")
                nc.tensor.matmul(
                    sAB[:, 0:L],
                    kTh[hA][:, j * CHUNK:(j + 1) * CHUNK],
                    qTh[hA][:, t0:t0 + L],
                    start=True, stop=True)
                nc.tensor.matmul(
                    sAB[:, SST:SST + L],
                    kTh[hB][:, j * CHUNK:(j + 1) * CHUNK],
                    qTh[hB][:, t0:t0 + L],
                    start=True, stop=True)
                eAB = wp.tile([CHUNK, 2 * SST], BF16, tag="eAB", bufs=3,
                              name=f"e_{p}_{ks}_{j}")
                if L == SST:
                    nc.scalar.activation(eAB, sAB, Exp)
                else:
                    nc.scalar.activation(eAB[:, 0:L], sAB[:, 0:L], Exp)
                    nc.scalar.activation(eAB[:, SST:SST + L],
                                         sAB[:, SST:SST + L], Exp)
                if CHUNK * j >= SST * ks:  # diagonal tile: zero upper triangle
                    for base in (0, SST):
                        nc.gpsimd.affine_select(
                            out=eAB[:, base:base + CHUNK],
                            in_=eAB[:, base:base + CHUNK],
                            pattern=[[1, CHUNK]], channel_multiplier=-1,
                            base=0, compare_op=mybir.AluOpType.is_ge, fill=0.0)
                av = av_cur[0]
                nc.tensor.matmul(
                    av[:, off:SST],
                    vaug[j][:, hA * VW:(hA + 1) * VW],
                    eAB[:, 0:L],
                    start=(j == 0), stop=(j == nj - 1), skip_group_check=True)
                nc.tensor.matmul(
                    av[:, SST + off:2 * SST],
                    vaug[j][:, hB * VW:(hB + 1) * VW],
                    eAB[:, SST:SST + L],
                    start=(j == 0), stop=(j == nj - 1), skip_group_check=True)

            def normalize(p, ks, tail=False):
                """softmax-normalize strip ks of pair p into aoT[p].

                av rows 0:64 carry the PE-replicated denominator, 64:128 the
                unnormalized output."""
                av = av_cur[0]
                strip = slice(ks * SST, (ks + 1) * SST)
                if tail:
                    # low-latency path: 1/d = exp(-log d) on the (idle) scalar
                    rec = wp.tile([CHUNK, SST], F32, tag="rec", bufs=2,
                                  name=f"rec_{p}_{ks}")
                    nc.vector.tensor_copy(rec[0:D, :], av[0:D, 0:SST])
                    nc.vector.tensor_copy(rec[D:CHUNK, :], av[0:D, SST:2 * SST])
                    nc.vector.tensor_copy(aoT[p][0:D, strip],
                                          av[D:CHUNK, 0:SST])
                    nc.vector.tensor_copy(aoT[p][D:CHUNK, strip],
                                          av[D:CHUNK, SST:2 * SST])
                    nc.scalar.activation(rec, rec,
                                         mybir.ActivationFunctionType.Ln)
                    nc.scalar.activation(rec, rec, Exp, scale=-1.0)
                    nc.gpsimd.tensor_mul(aoT[p][:, strip], aoT[p][:, strip], rec)
                    return
                den = wp.tile([1, 2 * SST], F32, tag="den", bufs=2,
                              name=f"den_{p}_{ks}")
                # evict (unnormalized) + denominator row; frees the av banks
                nc.vector.tensor_copy(den, av[0:1, :])
                nc.vector.tensor_copy(aoT[p][0:D, strip],
                                      av[D:CHUNK, 0:SST])
                if ks < 2:      # scalar has slack in the shallow strips
                    nc.scalar.activation(aoT[p][D:CHUNK, strip],
                                         av[D:CHUNK, SST:2 * SST],
                                         mybir.ActivationFunctionType.Copy)
                else:
                    nc.vector.tensor_copy(aoT[p][D:CHUNK, strip],
                                          av[D:CHUNK, SST:2 * SST])
                # reciprocal via a [128, 8] reshape (wide on the DVE lanes),
                # then partition-broadcast — both through small DRAM bounces
                # (SBUF APs cannot reshape across partitions / stride-0 bcast)
                dden = dr.tile([1, 2 * SST], F32, tag="dden", bufs=2,
                               name=f"dden_{p}_{ks}")
                nc.sync.dma_start(out=dden, in_=den)
                nf = 2 * SST // CHUNK
                dsb = wp.tile([CHUNK, nf], F32, tag="dsb", bufs=2,
                              name=f"dsb_{p}_{ks}")
                dview = dden.rearrange("a b -> (a b)").rearrange(
                    "(p f) -> p f", p=CHUNK)
                nc.sync.dma_start(out=dsb, in_=dview)
                rsb = wp.tile([CHUNK, nf], F32, tag="rsb", bufs=2,
                              name=f"rsb_{p}_{ks}")
                nc.vector.reciprocal(rsb, dsb)
                drec = dr.tile([1, 2 * SST], F32, tag="drec", bufs=2,
                               name=f"drec_{p}_{ks}")
                rview = drec.rearrange("a b -> (a b)").rearrange(
                    "(p f) -> p f", p=CHUNK)
                nc.sync.dma_start(out=rview, in_=rsb)
                bcf = wp.tile([CHUNK, SST], F32, tag="bcf", bufs=2,
                              name=f"bcf_{p}_{ks}")
                nc.sync.dma_start(
                    out=bcf[0:D, :],
                    in_=drec[0:1, 0:SST].broadcast_to((D, SST)))
                nc.sync.dma_start(
                    out=bcf[D:CHUNK, :],
                    in_=drec[0:1, SST:2 * SST].broadcast_to((D, SST)))
                nc.gpsimd.tensor_mul(aoT[p][:, strip], aoT[p][:, strip], bcf)

            # =========== schedule ===========
            # pre-loop: full projection of strip 0
            for pj in (1, 0):
                for m in range(2):
                    proj_qk_group(0, pj, m, xt_bufs[0])
            for u in range(4):
                proj_v_group(0, u, xt_bufs[0])

            av_cur = [None]
            for ks in range(nks):
                if ks == 0:
                    load_wo()
                # prefetch x strip ks+2 (its buffer was freed by proj(ks))
                if ks + 2 < nks:
                    xt_bufs[ks + 2] = load_xt(ks + 2, [4, 4], [nc.sync])

                fillers = []
                if ks + 1 < nks:                  # projection of next strip
                    xt = xt_bufs[ks + 1]
                    for pj in (1, 0):
                        for m in range(2):
                            fillers.append((proj_qk_group, (ks + 1, pj, m, xt)))
                    for u in range(4):
                        fillers.append((proj_v_group, (ks + 1, u, xt)))
                if ks == nks - 1:                 # deferred out-proj (strips 0..ks-1)
                    for jt in range(4 * (nks - 1)):
                        fillers.append((outproj_group, (jt,)))

                nj = 4 * ks + 4
                total_js = 2 * nj
                gi = 0
                cnt = 0
                for p in (0, 1):
                    av_cur[0] = ps.tile([CHUNK, 2 * SST], F32, tag="av", bufs=1,
                                        name=f"av_{p}_{ks}")
                    for j in range(nj):
                        jstep(p, ks, j, nj)
                        cnt += 1
                        want = (cnt * len(fillers)) // total_js
                        while gi < want:
                            fn, args = fillers[gi]
                            fn(*args)
                            gi += 1
                    normalize(p, ks, tail=(ks == nks - 1 and p == 1))
                    # keep the tensor queue fed while av drains
                    if gi < len(fillers):
                        fn, args = fillers[gi]
                        fn(*args)
                        gi += 1
                while gi < len(fillers):
                    fn, args = fillers[gi]
                    fn(*args)
                    gi += 1

            # tail: out-proj of the last strip
            for jt in range(4 * (nks - 1), 4 * nks):
                outproj_group(jt, tail=True)

    if fix_waits:
        split_excess_waits(nc)
    return nc


def make_in_maps(x, wq, bq, wk, bk, wv, bv, wo, bo, t_len=T):
    """Build the 8 per-core input dicts from full inputs."""
    in_maps = []
    scale = 1.0 / np.sqrt(np.float32(D))
    for core in range(NCORES):
        b, hg = core // 4, core % 4
        sl = slice(DPC * hg, DPC * (hg + 1))
        wqs = (wq[sl] * scale).astype(np.float32)
        bqs = (bq[sl] * scale).astype(np.float32)
        wqkvT = np.concatenate([wqs.T, wk[sl].T, wv[sl].T], axis=1)
        bqk = np.stack([bqs[0:CHUNK], bqs[CHUNK:2 * CHUNK],
                        bk[sl][0:CHUNK], bk[sl][CHUNK:2 * CHUNK]], axis=1)
        in_maps.append({
            "xT": np.ascontiguousarray(x[b, :t_len].T).astype(BF16_NP),
            "wqkvT": np.ascontiguousarray(wqkvT).astype(BF16_NP),
            "woT": np.ascontiguousarray(wo[:, sl].T).astype(BF16_NP),
            "bqk": np.ascontiguousarray(bqk, dtype=np.float32),
            "bv_row": np.ascontiguousarray(bv[sl][None, :], dtype=np.float32),
        })
    return in_maps


def gather_output(results, bo, t_len=T):
    ys = [np.asarray(results[i]["y"], dtype=np.float32) for i in range(NCORES)]
    out = np.stack([ys[0] + ys[1] + ys[2] + ys[3],
                    ys[4] + ys[5] + ys[6] + ys[7]])
    out += np.asarray(bo, np.float32)[None, None, :]
    return out


_NC_CACHE = {}


def _get_nc(t_len=T):
    if t_len not in _NC_CACHE:
        _NC_CACHE[t_len] = build_nc(t_len)
    return _NC_CACHE[t_len]


def kernel(x, wq, bq, wk, bk, wv, bv, wo, bo, mask=None, **_unused):
    """Full-input entry point: shard, run on 8 NeuronCores, gather."""
    from concourse.bass_utils import run_bass_kernel_spmd

    x = np.asarray(x, dtype=np.float32)
    in_maps = make_in_maps(x, np.asarray(wq, np.float32), np.asarray(bq, np.float32),
                           np.asarray(wk, np.float32), np.asarray(bk, np.float32),
                           np.asarray(wv, np.float32), np.asarray(bv, np.float32),
                           np.asarray(wo, np.float32), np.asarray(bo, np.float32))
    nc = _get_nc(T)
    res = run_bass_kernel_spmd(nc, in_maps, list(range(NCORES)))
    return gather_output(res.results, bo)
